# revision 5
# baseline (speedup 1.0000x reference)
"""DTNN layer kernel for Trainium2 (8 NeuronCores).

Math: out[b,i,o] = sum_j sum_h Wfc[o,h] * hx[b,i,h] * hd[b,i,j,h]
with hx = x@Wcf.T + bcf, hd = dist@Wdf.T + bdf.
Since Wfc/Wdf are linear, the j-sum commutes:
    ds[b,i,d]  = sum_j dist[b,i,j,d]                  (memory-bound reduction)
    out[b,i,:] = ((x@Wcf.T + bcf) * (ds@Wdf.T + N*bdf)) @ Wfc.T
So the kernel streams `distance` once and does a few 128x128 matmuls.

Sharding: flatten (B,N) -> 1024 i-rows, 128 rows per core; no cross-core comms.

v3 design (from NTFF trace analysis of v1/v2):
- distance is cast to fp16 on the host (tolerance is 2e-2; result stays at
  ~1e-3): halves the HBM stream and doubles DVE throughput.
- Host lays the per-core shard out as [d, j, i]; a partial in-SBUF halving
  fold over j yields 8 blocks of partial ds^T per tile.
- The j-sum commutes through Wdf, so the remaining reduction rides the
  (otherwise idle) PE: each 128-col block is one fp16 accumulating matmul
  into hd_ps.  This caps DVE busy (~21us) below the stream span (~24us);
  in v2 a full DVE fold (28.5us busy at the measured 0.625ns/elem TT rate)
  lagged the stream by ~13us.
- Constants ride FIRST on the stream HWDGE queue (a second queue gets
  starved by packet round-robin against big stream packets).
- Big 64-j tiles stream early so fold+matmul work overlaps arrival; a tiny
  4-j tile lands last so the post-stream serial tail is short.
- All-fp16 tail: hx^T is kept in fp16, the bias term uses a host-folded
  N*bdf[h]*Wfc[h,o] matrix, and the final out-matmul is fp16 single-pass
  (the v2 fp32 LOW_HIGH out-matmul cost two PE passes).
"""

import numpy as np

import concourse.bass as bass
import concourse.bacc as bacc
import concourse.mybir as mybir
from concourse.tile import TileContext
from concourse.bass_utils import run_bass_kernel_spmd

B, N, D, H = 4, 256, 128, 128
NCORES = 8
ROWS = B * N // NCORES  # 128 i-rows per core
FP = mybir.dt.float32
F16 = mybir.dt.float16

# dist DRAM layout per core: [128 d-partitions, N*ROWS cols], col = j*ROWS + i
# Uniform 32-j tiles: each folds to 8 blocks in 2 DVE ops (1.9us, under the
# 2.6us arrival spacing, so the fold never lags the stream), with only tiny
# 8/4-j tiles at the end to keep the post-stream serial tail short.
SIZES = [8, 32, 32, 32, 32, 32, 32, 32, 8, 8, 4, 4]  # j per tile (all pow2)

# cst16 columns (all fp16)
C16_XT = 0      # x^T            (128 d, ROWS i)
C16_WCF = 128   # Wcf^T          (128 d, H)
C16_WDF = 256   # Wdf^T          (128 d, H)
C16_BCF = 384   # partition 0: bcf row (1, H)
C16_ONES = 512  # partition 0: ones row (1, ROWS)
C16_WFC = 640   # Wfc^T          (128 h, D)
C16_WFCB = 768  # N*bdf[h] * Wfc^T[h,o]  (128 h, D)
C16_TOT = 896


def build_nc():
    nc = bacc.Bacc("TRN2", target_bir_lowering=False)
    dist = nc.declare_dram_parameter("dist", [128, N * ROWS], F16, isOutput=False)
    cst16 = nc.declare_dram_parameter("cst16", [128, C16_TOT], F16, isOutput=False)
    out = nc.declare_dram_parameter("out", [ROWS, D], FP, isOutput=True)

    with TileContext(nc) as tc:
        with (
            tc.tile_pool(name="const", bufs=1) as cpool,
            tc.tile_pool(name="dist", bufs=1) as dpool,
            tc.tile_pool(name="work", bufs=1) as wpool,
            tc.tile_pool(name="psum", bufs=1, space="PSUM") as ppool,
        ):
            # Constants first on the stream queue (~0.6us), then the big
            # dist tiles follow on the same HWDGE ring (in-order arrivals).
            c16 = cpool.tile([128, C16_TOT], F16)
            nc.sync.dma_start(out=c16[:], in_=cst16[:])

            dtiles = []
            off = 0
            for k, jn in enumerate(SIZES):
                t = dpool.tile([128, jn * ROWS], F16, tag=f"dist{k}")
                nc.sync.dma_start(out=t[:], in_=dist[:, off * ROWS:(off + jn) * ROWS])
                dtiles.append(t)
                off += jn

            xT = c16[:, C16_XT:C16_XT + ROWS]
            wcf = c16[:, C16_WCF:C16_WCF + H]
            wdf = c16[:, C16_WDF:C16_WDF + H]
            bcf_row = c16[0:1, C16_BCF:C16_BCF + H]
            ones_row = c16[0:1, C16_ONES:C16_ONES + ROWS]
            wfc16 = c16[:, C16_WFC:C16_WFC + D]
            wfcb16 = c16[:, C16_WFCB:C16_WFCB + D]

            # hx^T = (Wcf^T)^T @ x^T + bcf x ones -> (H, ROWS) in PSUM,
            # kept in fp16 for the all-fp16 tail matmuls.
            hx_ps = ppool.tile([H, ROWS], FP)
            nc.tensor.matmul(hx_ps[:], wcf, xT, start=True, stop=False)
            nc.tensor.matmul(hx_ps[:], bcf_row, ones_row, start=False, stop=True)
            hxT = wpool.tile([H, ROWS], F16)
            nc.vector.tensor_copy(hxT[:], hx_ps[:])

            # Preload the bias term hx^T @ (N*bdf*Wfc^T) into the output
            # PSUM during the stream; the tail's out-matmul accumulates on it.
            out_ps = ppool.tile([ROWS, D], FP)
            nc.tensor.matmul(out_ps[:], hxT[:], wfcb16, start=True, stop=False)

            # Streaming j-reduction: each tile [128 d, jn*ROWS] is jn blocks
            # of ROWS columns.  DVE halving adds (2x fp16 mode) fold big
            # tiles down to 8 blocks; each remaining block is one fp16
            # accumulating matmul into hd_ps on the otherwise-idle PE
            # (sum_j commutes through Wdf).  Small tiles fold to 1 block.
            hd_ps = ppool.tile([H, ROWS], FP)
            n_mms = sum(8 if jn >= 16 else 1 for jn in SIZES)
            mi = 0
            for k, jn in enumerate(SIZES):
                t = dtiles[k]
                nblk = 8 if jn >= 16 else 1
                half = jn // 2
                while half >= nblk:
                    nc.vector.tensor_add(
                        t[:, 0:half * ROWS],
                        t[:, 0:half * ROWS],
                        t[:, half * ROWS:2 * half * ROWS],
                    )
                    half //= 2
                for b in range(nblk):
                    nc.tensor.matmul(
                        hd_ps[:], wdf, t[:, b * ROWS:(b + 1) * ROWS],
                        start=(mi == 0), stop=(mi == n_mms - 1),
                    )
                    mi += 1

            # s^T = hd^T * hx^T (one PSUM operand max per DVE op), fp16
            sT = wpool.tile([H, ROWS], F16)
            nc.vector.tensor_mul(sT[:], hd_ps[:], hxT[:])

            # out += s^T^T @ Wfc^T (fp16 single pass), onto the bias term
            nc.tensor.matmul(out_ps[:], sT[:], wfc16, start=False, stop=True,
                             skip_group_check=True)
            out_sb = wpool.tile([ROWS, D], FP)
            nc.vector.tensor_copy(out_sb[:], out_ps[:])
            nc.sync.dma_start(out=out[:], in_=out_sb[:])
    nc.compile()
    return nc


_NC_CACHE = None


def _get_nc():
    global _NC_CACHE
    if _NC_CACHE is None:
        _NC_CACHE = build_nc()
    return _NC_CACHE


def _make_in_maps(x, distance, Wcf_w, Wcf_b, Wdf_w, Wdf_b, Wfc_w):
    x = np.asarray(x, np.float32)
    x_flat = x.reshape(B * N, D)
    # [B*N, N, D] -> fp16 -> [d, j, i_full] once, then slice per core
    d16 = np.asarray(distance, np.float32).astype(np.float16)
    dT = np.ascontiguousarray(d16.reshape(B * N, N, D).transpose(2, 1, 0))
    wcfT = np.asarray(Wcf_w, np.float32).T
    wdfT = np.asarray(Wdf_w, np.float32).T
    wfcT = np.asarray(Wfc_w, np.float32).T
    bcf = np.asarray(Wcf_b, np.float32)
    bdf = np.asarray(Wdf_b, np.float32)
    wfcb = (float(N) * bdf)[:, None] * wfcT  # (h, o)
    in_maps = []
    for c in range(NCORES):
        sl = slice(c * ROWS, (c + 1) * ROWS)
        c16blk = np.zeros((128, C16_TOT), np.float16)
        c16blk[:, C16_XT:C16_XT + ROWS] = x_flat[sl].T
        c16blk[:, C16_WCF:C16_WCF + H] = wcfT
        c16blk[:, C16_WDF:C16_WDF + H] = wdfT
        c16blk[0, C16_BCF:C16_BCF + H] = bcf
        c16blk[0, C16_ONES:C16_ONES + ROWS] = 1.0
        c16blk[:, C16_WFC:C16_WFC + D] = wfcT
        c16blk[:, C16_WFCB:C16_WFCB + D] = wfcb
        in_maps.append({
            "dist": np.ascontiguousarray(dT[:, :, sl]).reshape(128, N * ROWS),
            "cst16": c16blk,
        })
    return in_maps


def kernel(x, distance, Wcf_w, Wcf_b, Wdf_w, Wdf_b, Wfc_w):
    in_maps = _make_in_maps(x, distance, Wcf_w, Wcf_b, Wdf_w, Wdf_b, Wfc_w)
    nc = _get_nc()
    res = run_bass_kernel_spmd(nc, in_maps, list(range(NCORES))).results
    out = np.concatenate([res[c]["out"] for c in range(NCORES)], axis=0)
    return out.reshape(B, N, D)


# revision 6
# speedup vs baseline: 1.0056x; 1.0056x over previous
"""DTNN layer kernel for Trainium2 (8 NeuronCores).

Math: out[b,i,o] = sum_j sum_h Wfc[o,h] * hx[b,i,h] * hd[b,i,j,h]
with hx = x@Wcf.T + bcf, hd = dist@Wdf.T + bdf.
Since Wfc/Wdf are linear, the j-sum commutes:
    ds[b,i,d]  = sum_j dist[b,i,j,d]                  (memory-bound reduction)
    out[b,i,:] = ((x@Wcf.T + bcf) * (ds@Wdf.T + N*bdf)) @ Wfc.T
So the kernel streams `distance` once and does a few 128x128 matmuls.

Sharding: flatten (B,N) -> 1024 i-rows, 128 rows per core; no cross-core comms.

v3 design (from NTFF trace analysis of v1/v2):
- distance is cast to fp16 on the host (tolerance is 2e-2; result stays at
  ~1e-3): halves the HBM stream and doubles DVE throughput.
- Host lays the per-core shard out as [d, j, i]; a partial in-SBUF halving
  fold over j yields 8 blocks of partial ds^T per tile.
- The j-sum commutes through Wdf, so the remaining reduction rides the
  (otherwise idle) PE: each 128-col block is one fp16 accumulating matmul
  into hd_ps.  This caps DVE busy (~21us) below the stream span (~24us);
  in v2 a full DVE fold (28.5us busy at the measured 0.625ns/elem TT rate)
  lagged the stream by ~13us.
- Constants ride FIRST on the stream HWDGE queue (a second queue gets
  starved by packet round-robin against big stream packets).
- Big 64-j tiles stream early so fold+matmul work overlaps arrival; a tiny
  4-j tile lands last so the post-stream serial tail is short.
- All-fp16 tail: hx^T is kept in fp16, the bias term uses a host-folded
  N*bdf[h]*Wfc[h,o] matrix, and the final out-matmul is fp16 single-pass
  (the v2 fp32 LOW_HIGH out-matmul cost two PE passes).
"""

import numpy as np

import concourse.bass as bass
import concourse.bacc as bacc
import concourse.mybir as mybir
from concourse.tile import TileContext
from concourse.bass_utils import run_bass_kernel_spmd

B, N, D, H = 4, 256, 128, 128
NCORES = 8
ROWS = B * N // NCORES  # 128 i-rows per core
FP = mybir.dt.float32
F16 = mybir.dt.float16

# dist DRAM layout per core: [128 d-partitions, N*ROWS cols], col = j*ROWS + i
# Big 64-j tiles early (fold+matmul overlap their arrival), small tiles last
# so the post-stream serial tail is short.  A finer 12x32-j split was tried
# and measured neutral-to-worse (more DMAs, smaller packets).
SIZES = [4, 64, 64, 64, 32, 16, 8, 4]  # j-counts per DMA tile (all pow2)

# cst16 columns (all fp16)
C16_XT = 0      # x^T            (128 d, ROWS i)
C16_WCF = 128   # Wcf^T          (128 d, H)
C16_WDF = 256   # Wdf^T          (128 d, H)
C16_BCF = 384   # partition 0: bcf row (1, H)
C16_ONES = 512  # partition 0: ones row (1, ROWS)
C16_WFC = 640   # Wfc^T          (128 h, D)
C16_WFCB = 768  # N*bdf[h] * Wfc^T[h,o]  (128 h, D)
C16_TOT = 896


def build_nc():
    nc = bacc.Bacc("TRN2", target_bir_lowering=False)
    dist = nc.declare_dram_parameter("dist", [128, N * ROWS], F16, isOutput=False)
    cst16 = nc.declare_dram_parameter("cst16", [128, C16_TOT], F16, isOutput=False)
    out = nc.declare_dram_parameter("out", [ROWS, D], FP, isOutput=True)

    with TileContext(nc) as tc:
        with (
            tc.tile_pool(name="const", bufs=1) as cpool,
            tc.tile_pool(name="dist", bufs=1) as dpool,
            tc.tile_pool(name="work", bufs=1) as wpool,
            tc.tile_pool(name="psum", bufs=1, space="PSUM") as ppool,
        ):
            # Constants first on the stream queue (~0.6us), then the big
            # dist tiles follow on the same HWDGE ring (in-order arrivals).
            c16 = cpool.tile([128, C16_TOT], F16)
            nc.sync.dma_start(out=c16[:], in_=cst16[:])

            dtiles = []
            off = 0
            for k, jn in enumerate(SIZES):
                t = dpool.tile([128, jn * ROWS], F16, tag=f"dist{k}")
                nc.sync.dma_start(out=t[:], in_=dist[:, off * ROWS:(off + jn) * ROWS])
                dtiles.append(t)
                off += jn

            xT = c16[:, C16_XT:C16_XT + ROWS]
            wcf = c16[:, C16_WCF:C16_WCF + H]
            wdf = c16[:, C16_WDF:C16_WDF + H]
            bcf_row = c16[0:1, C16_BCF:C16_BCF + H]
            ones_row = c16[0:1, C16_ONES:C16_ONES + ROWS]
            wfc16 = c16[:, C16_WFC:C16_WFC + D]
            wfcb16 = c16[:, C16_WFCB:C16_WFCB + D]

            # hx^T = (Wcf^T)^T @ x^T + bcf x ones -> (H, ROWS) in PSUM,
            # kept in fp16 for the all-fp16 tail matmuls.
            hx_ps = ppool.tile([H, ROWS], FP)
            nc.tensor.matmul(hx_ps[:], wcf, xT, start=True, stop=False)
            nc.tensor.matmul(hx_ps[:], bcf_row, ones_row, start=False, stop=True)
            hxT = wpool.tile([H, ROWS], F16)
            nc.vector.tensor_copy(hxT[:], hx_ps[:])

            # Preload the bias term hx^T @ (N*bdf*Wfc^T) into the output
            # PSUM during the stream; the tail's out-matmul accumulates on it.
            out_ps = ppool.tile([ROWS, D], FP)
            nc.tensor.matmul(out_ps[:], hxT[:], wfcb16, start=True, stop=False)

            # Streaming j-reduction: each tile [128 d, jn*ROWS] is jn blocks
            # of ROWS columns.  DVE halving adds (2x fp16 mode) fold big
            # tiles down to 8 blocks; each remaining block is one fp16
            # accumulating matmul into hd_ps on the otherwise-idle PE
            # (sum_j commutes through Wdf).  Small tiles fold to 1 block.
            hd_ps = ppool.tile([H, ROWS], FP)
            n_mms = sum(8 if jn >= 16 else 1 for jn in SIZES)
            mi = 0
            for k, jn in enumerate(SIZES):
                t = dtiles[k]
                nblk = 8 if jn >= 16 else 1
                half = jn // 2
                while half >= nblk:
                    nc.vector.tensor_add(
                        t[:, 0:half * ROWS],
                        t[:, 0:half * ROWS],
                        t[:, half * ROWS:2 * half * ROWS],
                    )
                    half //= 2
                for b in range(nblk):
                    nc.tensor.matmul(
                        hd_ps[:], wdf, t[:, b * ROWS:(b + 1) * ROWS],
                        start=(mi == 0), stop=(mi == n_mms - 1),
                    )
                    mi += 1

            # s^T = hd^T * hx^T (one PSUM operand max per DVE op), fp16
            sT = wpool.tile([H, ROWS], F16)
            nc.vector.tensor_mul(sT[:], hd_ps[:], hxT[:])

            # out += s^T^T @ Wfc^T (fp16 single pass), onto the bias term
            nc.tensor.matmul(out_ps[:], sT[:], wfc16, start=False, stop=True,
                             skip_group_check=True)
            out_sb = wpool.tile([ROWS, D], FP)
            nc.vector.tensor_copy(out_sb[:], out_ps[:])
            nc.sync.dma_start(out=out[:], in_=out_sb[:])
    nc.compile()
    return nc


_NC_CACHE = None


def _get_nc():
    global _NC_CACHE
    if _NC_CACHE is None:
        _NC_CACHE = build_nc()
    return _NC_CACHE


def _make_in_maps(x, distance, Wcf_w, Wcf_b, Wdf_w, Wdf_b, Wfc_w):
    x = np.asarray(x, np.float32)
    x_flat = x.reshape(B * N, D)
    # [B*N, N, D] -> fp16 -> [d, j, i_full] once, then slice per core
    d16 = np.asarray(distance, np.float32).astype(np.float16)
    dT = np.ascontiguousarray(d16.reshape(B * N, N, D).transpose(2, 1, 0))
    wcfT = np.asarray(Wcf_w, np.float32).T
    wdfT = np.asarray(Wdf_w, np.float32).T
    wfcT = np.asarray(Wfc_w, np.float32).T
    bcf = np.asarray(Wcf_b, np.float32)
    bdf = np.asarray(Wdf_b, np.float32)
    wfcb = (float(N) * bdf)[:, None] * wfcT  # (h, o)
    in_maps = []
    for c in range(NCORES):
        sl = slice(c * ROWS, (c + 1) * ROWS)
        c16blk = np.zeros((128, C16_TOT), np.float16)
        c16blk[:, C16_XT:C16_XT + ROWS] = x_flat[sl].T
        c16blk[:, C16_WCF:C16_WCF + H] = wcfT
        c16blk[:, C16_WDF:C16_WDF + H] = wdfT
        c16blk[0, C16_BCF:C16_BCF + H] = bcf
        c16blk[0, C16_ONES:C16_ONES + ROWS] = 1.0
        c16blk[:, C16_WFC:C16_WFC + D] = wfcT
        c16blk[:, C16_WFCB:C16_WFCB + D] = wfcb
        in_maps.append({
            "dist": np.ascontiguousarray(dT[:, :, sl]).reshape(128, N * ROWS),
            "cst16": c16blk,
        })
    return in_maps


def kernel(x, distance, Wcf_w, Wcf_b, Wdf_w, Wdf_b, Wfc_w):
    in_maps = _make_in_maps(x, distance, Wcf_w, Wcf_b, Wdf_w, Wdf_b, Wfc_w)
    nc = _get_nc()
    res = run_bass_kernel_spmd(nc, in_maps, list(range(NCORES))).results
    out = np.concatenate([res[c]["out"] for c in range(NCORES)], axis=0)
    return out.reshape(B, N, D)
